# revision 6
# baseline (speedup 1.0000x reference)
"""DGCNN forward kernel for Trainium2, 8 NeuronCores, pure data parallelism.

Math (per graph b):  h_b = relu(sum_k T_k @ X_b @ W_k);  out_b = h_b.flat @ fc_w + fc_b
with T_k the (symmetric) Chebyshev supports of the normalized adjacency.

Device pipeline (per core, 512 graphs):
  stage A: per batch-pair, lhsT = x_pair [128=(j,m), 128=c] (natural DRAM block),
           rhs = blockdiag(T_all, T_all) [128, 512=(j,k,n)] -> PSUM Q^T[c,(j,k,n)]
  stage B: per 4-pair group, lhsT = W_k [c,o], rhs = Q^T slices, PSUM-accumulate
           over k -> h^T[o, (pair,j,n)]
  relu   : ACT copy PSUM->SBUF
  stage C: per n (64 matmuls), lhsT = F_n [o,3], rhs = h^T[:, (batches, n)],
           PSUM-accumulate over n -> logits^T [3, batches]
All matmuls run as float32r (1 cy/row at free dim >= 256).
"""

import sys

if "/opt/trn_rl_repo" not in sys.path:
    sys.path.insert(0, "/opt/trn_rl_repo")

from contextlib import ExitStack

import numpy as np

import concourse.bass as bass
import concourse.tile as tile
from concourse import bacc
from concourse import mybir
from concourse.bass_utils import run_bass_kernel_spmd

B, N, C, K, O, CLS = 4096, 64, 128, 4, 64, 3
NCORES = 8

f32 = mybir.dt.float32
f32r = mybir.dt.float32r


def _host_constants(A, gc_w, fc_w):
    """Mirror reference.py's normalize_A + Chebyshev supports in fp32 numpy."""
    eye = np.eye(N, dtype=np.float32)
    Ar = np.maximum(A, 0) * (1 - eye)
    Ar = Ar + Ar.T
    d = (1.0 / np.sqrt(Ar.sum(axis=1) + 1e-10)).astype(np.float32)
    L = eye - (d[:, None] * Ar) * d[None, :]
    Ln = (L - eye).astype(np.float32)  # 2L/2 - I
    sup = [eye, Ln]
    for _ in range(2, K):
        sup.append(2.0 * Ln @ sup[-1] - sup[-2])
    T = np.stack(sup[:K]).astype(np.float32)  # [K,N,N], each symmetric

    T_all = np.zeros((N, K * N), np.float32)  # [m,(k,n)]
    for k in range(K):
        T_all[:, k * N : (k + 1) * N] = T[k]
    BDT = np.zeros((2 * N, 2 * K * N), np.float32)  # [(j,m),(j,k,n)]
    BDT[:N, : K * N] = T_all
    BDT[N:, K * N :] = T_all
    W_sb = np.concatenate([gc_w[k] for k in range(K)], axis=1)  # [c,(k,o)]
    F_sb = np.ascontiguousarray(fc_w.reshape(N, O, CLS).transpose(1, 0, 2)).reshape(
        O, N * CLS
    )  # [o,(n,cls)]
    return BDT, W_sb, F_sb


def build(bs, gp=4, sg_count=2, xp_bufs=3, qp_bufs=2, psa_bufs=3, psb_bufs=2, psc_bufs=2, hp_bufs=2):
    """Build the SPMD Bass program for a per-core shard of `bs` graphs."""
    pairs = bs // 2
    ngroups = pairs // gp
    gps = ngroups // sg_count  # groups per supergroup
    sgb = gps * gp * 2  # batches per supergroup

    nc = bacc.Bacc()
    x_in = nc.declare_dram_parameter("x", [bs * N, C], f32, isOutput=False)
    bdt_in = nc.declare_dram_parameter("bdt", [2 * N, 2 * K * N], f32, isOutput=False)
    w_in = nc.declare_dram_parameter("w", [C, K * O], f32, isOutput=False)
    f_in = nc.declare_dram_parameter("f", [O, N * CLS], f32, isOutput=False)
    b_in = nc.declare_dram_parameter("b", [CLS, 1], f32, isOutput=False)
    out_ext = nc.declare_dram_parameter("out", [sg_count, CLS, sgb], f32, isOutput=True)

    with ExitStack() as ctx:
        tc = ctx.enter_context(tile.TileContext(nc))
        consts = ctx.enter_context(tc.tile_pool(name="consts", bufs=1))
        xp = ctx.enter_context(tc.tile_pool(name="xp", bufs=xp_bufs))
        qp = ctx.enter_context(tc.tile_pool(name="qp", bufs=qp_bufs))
        hp = ctx.enter_context(tc.tile_pool(name="hp", bufs=hp_bufs))
        outp = ctx.enter_context(tc.tile_pool(name="outp", bufs=2))
        psA = ctx.enter_context(tc.tile_pool(name="psA", bufs=psa_bufs, space="PSUM"))
        psB = ctx.enter_context(tc.tile_pool(name="psB", bufs=psb_bufs, space="PSUM"))
        psC = ctx.enter_context(tc.tile_pool(name="psC", bufs=psc_bufs, space="PSUM"))

        bdt_t = consts.tile([2 * N, 2 * K * N], f32r)
        nc.sync.dma_start(out=bdt_t, in_=bdt_in[:].bitcast(f32r))
        w_t = consts.tile([C, K * O], f32r)
        nc.sync.dma_start(out=w_t, in_=w_in[:].bitcast(f32r))
        f_t = consts.tile([O, N, CLS], f32r)
        nc.sync.dma_start(out=f_t, in_=f_in[:].rearrange("o (n cls) -> o n cls", cls=CLS).bitcast(f32r))
        bias_t = consts.tile([CLS, 1], f32)
        nc.sync.dma_start(out=bias_t, in_=b_in[:])

        x_view = x_in[:].rearrange("(g pair p) c -> g p pair c", p=2 * N, pair=gp)

        for sg in range(sg_count):
            h_t = hp.tile([O, gps, gp, 2, N], f32r)
            for g in range(gps):
                gg = sg * gps + g
                x_t = xp.tile([2 * N, gp, C], f32r)
                nc.sync.dma_start(out=x_t, in_=x_view[gg].bitcast(f32r))
                q_t = qp.tile([C, gp, 2, K, N], f32r)
                for p in range(gp):
                    qa_t = psA.tile([C, 2 * K * N], f32)
                    nc.tensor.matmul(
                        out=qa_t,
                        lhsT=x_t[:, p, :],
                        rhs=bdt_t[:],
                        start=True,
                        stop=True,
                    )
                    eng = nc.vector if p % 2 == 0 else nc.scalar
                    if p % 2 == 0:
                        eng.tensor_copy(out=q_t[:, p], in_=qa_t)
                    else:
                        eng.copy(out=q_t[:, p], in_=qa_t)
                hb_t = psB.tile([O, gp, 2, N], f32)
                for k in range(K):
                    nc.tensor.matmul(
                        out=hb_t,
                        lhsT=w_t[:, k * O : (k + 1) * O],
                        rhs=q_t[:, :, :, k, :],
                        start=(k == 0),
                        stop=(k == K - 1),
                    )
                nc.scalar.activation(
                    out=h_t[:, g],
                    in_=hb_t,
                    func=mybir.ActivationFunctionType.Relu,
                )
            oc_t = psC.tile([CLS, sgb], f32)
            for n in range(N):
                nc.tensor.matmul(
                    out=oc_t,
                    lhsT=f_t[:, n, :],
                    rhs=h_t[:, :, :, :, n],
                    start=(n == 0),
                    stop=(n == N - 1),
                )
            ost = outp.tile([CLS, sgb], f32)
            nc.vector.tensor_scalar_add(out=ost, in0=oc_t, scalar1=bias_t[:])
            nc.sync.dma_start(out=out_ext[sg], in_=ost)

    nc.compile()
    return nc


def run(x, A, gc_w, fc_w, fc_b, trace=False):
    x = np.ascontiguousarray(x, dtype=np.float32)
    BDT, W_sb, F_sb = _host_constants(
        np.asarray(A, np.float32), np.asarray(gc_w, np.float32), np.asarray(fc_w, np.float32)
    )
    bias = np.asarray(fc_b, np.float32).reshape(CLS, 1)

    bs = B // NCORES
    nc = build(bs)
    shards = x.reshape(NCORES, bs * N, C)
    in_maps = [
        {"x": shards[i], "bdt": BDT, "w": W_sb, "f": F_sb, "b": bias}
        for i in range(NCORES)
    ]
    br = run_bass_kernel_spmd(nc, in_maps, list(range(NCORES)), trace=trace)
    outs = []
    for i in range(NCORES):
        o = br.results[i]["out"]  # [sg, CLS, sgb]
        outs.append(np.concatenate([o[s].T for s in range(o.shape[0])], axis=0))
    return np.concatenate(outs, axis=0).astype(np.float32), br


def kernel(x, A, gc_w, fc_w, fc_b):
    out, _ = run(x, A, gc_w, fc_w, fc_b)
    return out


if __name__ == "__main__":
    rng = np.random.default_rng(0)
    x = rng.standard_normal((B, N, C), dtype=np.float32)
    A = rng.uniform(0.01, 0.5, (N, N)).astype(np.float32)
    gc_w = (rng.standard_normal((K, C, O), dtype=np.float32) * 0.1).astype(np.float32)
    fc_w = (rng.standard_normal((N * O, CLS), dtype=np.float32) * 0.02).astype(np.float32)
    fc_b = np.zeros(CLS, np.float32)
    out = kernel(x=x, A=A, gc_w=gc_w, fc_w=fc_w, fc_b=fc_b)
    print(out.shape, out.dtype)
